# revision 80
# baseline (speedup 1.0000x reference)
#!/usr/bin/env python3
"""Bass/Trainium2 kernel for nn_Attention_12747462934680.

Reference computation (B=64, L=2048, H=512):
    x = concat([hidden broadcast over L, encoder_outputs], -1)   # [B, L, 2H]
    energy = tanh(x @ W.T + b)                                   # [B, L, H]
    scores = energy @ v                                          # [B, L]
    attn = softmax(scores, axis=1)[:, None, :]                   # [B, 1, L]

Decomposition: pre[b,l,h] = (enc[b,l] @ W2.T)[h] + h1[b,h], with
W1 = W[:, :H], W2 = W[:, H:], h1 = hidden @ W1.T + bias (h1 is tiny and
computed on the host in f64, shipped as an fp16 input packed with W2T, v).

Sharding: data-parallel over B across 8 cores (8 batches/core).

Host prep: enc is cast to fp16 and laid out per core as [G=32 groups x
128 partitions x 2048] with token = n*128 + p inside each group.  This
halves HBM traffic and removes any on-device producer for the transpose
stage, so the DMA pipeline has no cross-queue dependencies for the tile
scheduler to serialize (on-device per-group load->transpose chains get
lock-stepped by the scheduler's coarsened cross-queue sem waits).

Per-core device pipeline (SPMD, no collectives), data path in fp16:
  - ~190 tiny warm-up matmuls keep the PE busy from t=0 so the p-state
    model reaches full clock before the first real matmul
  - loop over 32 groups of 512 tokens (4 groups per batch, b = i//4):
      XBAR DMA transpose (dma_start_transpose, SP HWDGE queue) reads the
      fp16 DRAM input directly -> st[k_lo, (n, kt), p]; per-kt slices
      have free dims (n, p) = token order exactly.  No PE transposes,
      no DVE PSUM copies, no separate load stage.
      preT[h, t] = W2T.T @ st (fp16 matmul, f32 PSUM; 16 x 512 rows =
      100% PE utilization)
      energy = tanh(preT + h1[b]) on ACT (PSUM -> SBUF, fp16)
      v-dot with energy as the STATIONARY operand: out[t, 1] accumulates
      v over the 4 h-tiles into one persistent PSUM tile
      scAll[128 t, 128 (b, j, n)] - each matmul streams only 1 row
      (vs 512 for a one-hot moving-operand formulation)
  - tail softmax without max-subtraction (scores are bounded ~ +-40, so
    exp is safe in f32), split in batch-halves: scAll cols 64h..64h+64
    hold batches 4h..4h+3 exactly, so half 0 (including its 64-row
    output DMA) runs hidden under groups 16+; the end-of-kernel tail is
    only half 1: PE f32-transpose of scAll -> scT[(b,j,n), t] at PSUM
    partition 0 (a HW requirement), ACT exp with accum_out giving
    per-(b,j,n) sums, one 1-row matmul with the block-sum-replicate
    matrix M2 (M2[p,m] = 1 iff p//16 == m//16) that turns row sums
    directly into per-partition batch totals, DVE reciprocal + scale,
    64-row output DMA.
"""
import sys
import numpy as np

sys.path.insert(0, "/opt/trn_rl_repo")

B, L, H = 64, 2048, 512
NCORES = 8
BPC = B // NCORES          # batches per core
T = BPC * L                # tokens per core = 16384
GT = 512                   # tokens per group
G = T // GT                # 32 groups
NJ = L // GT               # 4 l-chunks per batch
KT = H // 128              # 4 k-tiles
HT = H // 128              # 4 h-tiles
NT = GT // 128             # 4 t-tiles per group

_compiled = None


def _build(variant="full"):
    from contextlib import ExitStack
    from concourse import bacc, mybir
    import concourse.tile as tile
    from concourse.bass import ts

    f32 = mybir.dt.float32
    fp16 = mybir.dt.float16
    DT = fp16
    ActF = mybir.ActivationFunctionType

    nc = bacc.Bacc("TRN2", target_bir_lowering=False, debug=False,
                   enable_asserts=True, num_devices=NCORES)

    enc16_d = nc.dram_tensor("enc16", [G * 128, NT * H], fp16,
                             kind="ExternalInput").ap()
    wv_d = nc.dram_tensor("wv", [128, KT * H + KT + HT * BPC], fp16,
                          kind="ExternalInput").ap()
    m2_d = nc.dram_tensor("m2", [64, 64], f32, kind="ExternalInput").ap()
    identf_d = nc.dram_tensor("identf", [128, 128], f32,
                              kind="ExternalInput").ap()
    attn_d = nc.dram_tensor("attn", [BPC, L], f32, kind="ExternalOutput").ap()

    with tile.TileContext(nc) as tc:
        with ExitStack() as ctx:
            singles = ctx.enter_context(tc.tile_pool(name="singles", bufs=1))
            enctp = ctx.enter_context(tc.tile_pool(name="enctp", bufs=24))
            enrgp = ctx.enter_context(tc.tile_pool(name="enrgp", bufs=16))
            smp = ctx.enter_context(tc.tile_pool(name="smp", bufs=2))
            psP = ctx.enter_context(tc.tile_pool(name="psP", bufs=5, space="PSUM"))
            psS = ctx.enter_context(tc.tile_pool(name="psS", bufs=1, space="PSUM"))
            psT = ctx.enter_context(tc.tile_pool(name="psT", bufs=1, space="PSUM"))

            # ---- params ----
            # the sync (SP) HWDGE queue carries ONLY the XBAR transposes so
            # all 8 DMAHW sem lanes belong to them (param DMAs on HWDGE lanes
            # chained the first transposes to the params' late consumers);
            # params ride the otherwise-idle gpsimd SWDGE queue.
            wv_sb = singles.tile([128, KT * H + KT + HT * BPC], DT, tag="wv")
            vcol_sb = wv_sb[:, KT * H:KT * H + KT]
            h1b_sb = wv_sb[:, KT * H + KT:].rearrange(
                "p (ht b) -> p ht b", ht=HT)
            m2_sb = singles.tile([64, 64], f32, tag="m2")
            identf_sb = singles.tile([128, 128], f32, tag="identf")
            nc.sync.dma_start(out=wv_sb, in_=wv_d)


            def w2t_slice(kt, hsl):
                return wv_sb[:, kt * H:kt * H + H][:, hsl]

            # persistent score accumulator: [t within tile, (b, j, n)];
            # the bank's tail 64 columns double as the warm-up target
            scS = psS.tile([128, G * NT + 64], f32, tag="scAll",
                           name="scAll")
            scAll = scS[:, 0:G * NT]  # [128, 128]
            ps_warm = scS[:, G * NT:]

            # PE warm-up: keep the tensor engine busy from t=0 until the
            # first real matmul so the p-state model reaches full clock
            # (idle-dispatched matmuls are charged the slow-clock rate).
            warm_sb = singles.tile([128, 128], DT, tag="warm")
            nc.vector.memset(warm_sb, 0.0)
            for _ in range(120):
                nc.tensor.matmul(ps_warm, warm_sb, warm_sb[:, 0:64],
                                 start=True, stop=True, skip_group_check=True)

            # ---- main pipeline over 32 groups ----
            # Stage A (gpsimd SWDGE, casts): enc f32 [T, H] -> DRAM scratch
            # fp16 [G, 128, (n k)] in a few chunks (token = n*128 + p within
            # each group), each chunk .then_inc'ing a manual semaphore --
            # DRAM is not tile-managed, so the cast->transpose dependency is
            # expressed with explicit wait_ge at exact chunk granularity
            # (the tile scheduler's coarsened cross-queue sems serialized
            # the old per-group load->transpose chain into lock-step).
            # Stage B (SP HWDGE): XBAR transpose DRAM fp16 -> SBUF
            # st[c % 128, c // 128, p]: viewed as [k_lo, (n, kt), p], per-kt
            # free dims (n, p) = token order exactly.
            enc16_r = enc16_d.rearrange("(g p) c -> g p c", p=128)

            encT_tiles = {}
            energy_tiles = {}

            def stage_transpose(i):
                st = enctp.tile([128, NT, KT, 128], DT, tag="enct")
                if variant in ("notrans", "nodma"):
                    nc.vector.memset(st[:, 0, 0, 0:1], 0.0)
                elif i == 0:
                    # group 0 transposed in four 512-col chunks so the first
                    # matmuls can start ~2.4us earlier during pipeline fill
                    for c in range(NT):
                        nc.sync.dma_start_transpose(
                            out=st[:, c], in_=enc16_r[i][:, 512 * c:512 * c + 512])
                else:
                    nc.sync.dma_start_transpose(
                        out=st.rearrange("p n kt t -> p (n kt) t"),
                        in_=enc16_r[i])
                encT_tiles[i] = st

            def stage_mm(i):
                b = i // NJ
                st = encT_tiles.pop(i)
                energies = []
                for ht in range(HT):
                    ps_pre = psP.tile([128, GT], f32, tag="pspre")
                    if i == 0:
                        # n-chunked to consume group 0's chunked transposes
                        for n in range(NT):
                            for kt in range(KT):
                                nc.tensor.matmul(
                                    ps_pre[:, ts(n, 128)],
                                    w2t_slice(kt, ts(ht, 128)),
                                    st[:, n, kt, :],
                                    start=(kt == 0), stop=(kt == KT - 1),
                                    skip_group_check=True)
                    else:
                        for kt in range(KT):
                            nc.tensor.matmul(ps_pre, w2t_slice(kt, ts(ht, 128)),
                                             st[:, :, kt, :],
                                             start=(kt == 0),
                                             stop=(kt == KT - 1))
                    en = enrgp.tile([128, GT], DT, tag="energy")
                    nc.scalar.activation(out=en, in_=ps_pre, func=ActF.Tanh,
                                         bias=h1b_sb[:, ht, b:b + 1], scale=1.0)
                    energies.append(en)
                energy_tiles[i] = energies

            def stage_vdot(i):
                energies = energy_tiles.pop(i)
                if variant == "novdot":
                    return
                for n in range(NT):
                    col = i * NT + n
                    for ht in range(HT):
                        nc.tensor.matmul(scAll[:, col:col + 1],
                                         energies[ht][:, ts(n, 128)],
                                         vcol_sb[:, ht:ht + 1],
                                         start=(ht == 0), stop=(ht == HT - 1),
                                         skip_group_check=True)

            # tail softmax, split in batch-halves: scAll cols 64h..64h+64
            # hold groups 16h..16h+15 = batches 4h..4h+3 exactly, so half 0
            # (incl. its 64-row output DMA) runs as soon as vdot(15) is done,
            # hidden under groups 16+; the end-of-kernel tail is only half 1.
            # All matmul outputs sit at PSUM partition 0 (HW requirement).
            sc_sb = smp.tile([128, 128], f32, tag="sc_sb")
            ps_tail = psT.tile([128, 136], f32, tag="pstail")
            attn_r = attn_d.rearrange("b (j n t) -> (b j n) t", j=NJ, n=NT)

            def emit_tail_half(h):
                if variant == "novdot":
                    return
                cs = ts(h, 64)
                nc.vector.tensor_copy(sc_sb[:, cs], scAll[:, cs])
                scT = ps_tail[0:64, 0:128]
                nc.tensor.matmul(scT, sc_sb[:, cs], identf_sb,
                                 is_transpose=True, start=True, stop=True,
                                 skip_group_check=True)
                expT = smp.tile([64, 128], f32, tag="expTh")
                rowsum = smp.tile([64, 1], f32, tag="rowsumh")
                nc.scalar.activation(out=expT, in_=scT, func=ActF.Exp,
                                     scale=1.0, accum_out=rowsum)
                # one matmul with the block-sum-replicate matrix M2
                # (M2[p, m] = 1 iff p//16 == m//16) turns per-(j,n) row sums
                # directly into per-partition batch totals
                rsums = ps_tail[0:64, 131 + h:132 + h]
                nc.tensor.matmul(rsums, m2_sb, rowsum,
                                 start=True, stop=True, skip_group_check=True)
                rinv = smp.tile([64, 1], f32, tag="rinvh")
                nc.vector.reciprocal(rinv, rsums)
                attnT = smp.tile([64, 128], f32, tag="attnTh")
                nc.vector.tensor_scalar_mul(attnT, expT, rinv[:, 0:1])
                nc.sync.dma_start(out=attn_r[64 * h:64 * h + 64], in_=attnT)

            def emit_tail():
                pass

            for it in range(G + 1):
                if it < G:
                    stage_transpose(it)
                    stage_mm(it)
                if it >= 1:
                    stage_vdot(it - 1)
                if it == 10:
                    # tail constants loaded mid-loop: late enough that their
                    # DMA-order sems can't entangle the early transposes,
                    # early enough for the half-0 tail at it==17
                    nc.sync.dma_start(out=identf_sb, in_=identf_d)
                    nc.sync.dma_start(out=m2_sb, in_=m2_d)
                if it == G // 2 + 1:
                    emit_tail_half(0)
            emit_tail_half(1)
            emit_tail()

    nc.compile()
    return nc


class _Runner:
    """Compile once; jit once; run many times (mirrors run_bass_via_pjrt)."""

    def __init__(self):
        import jax
        import concourse.mybir as mybir
        from concourse.bass2jax import (_bass_exec_p, install_neuronx_cc_hook,
                                        partition_id_tensor)
        from jax.sharding import Mesh, PartitionSpec
        from jax.experimental.shard_map import shard_map

        install_neuronx_cc_hook()
        nc = _build()
        self.nc = nc

        in_names, out_names, out_avals = [], [], []
        for alloc in nc.m.functions[0].allocations:
            if not isinstance(alloc, mybir.MemoryLocationSet):
                continue
            name = alloc.memorylocations[0].name
            if alloc.kind == "ExternalInput":
                in_names.append(name)
            elif alloc.kind == "ExternalOutput":
                out_names.append(name)
                out_avals.append(jax.core.ShapedArray(
                    tuple(alloc.tensor_shape), mybir.dt.np(alloc.dtype)))
        part_name = (nc.partition_id_tensor.name
                     if nc.partition_id_tensor is not None else None)
        if part_name is not None and part_name in in_names:
            in_names.remove(part_name)
        self.in_names, self.out_names, self.out_avals = in_names, out_names, out_avals
        n_params = len(in_names)
        n_outs = len(out_names)
        all_names = in_names + out_names
        if part_name is not None:
            all_names = all_names + [part_name]

        def _body(*args):
            operands = list(args)
            if part_name is not None:
                operands.append(partition_id_tensor())
            return tuple(_bass_exec_p.bind(
                *operands,
                out_avals=tuple(out_avals),
                in_names=tuple(all_names),
                out_names=tuple(out_names),
                lowering_input_output_aliases=(),
                sim_require_finite=True,
                sim_require_nnan=True,
                nc=nc,
            ))

        devices = jax.devices()[:NCORES]
        self.mesh = Mesh(np.asarray(devices), ("core",))
        in_specs = (PartitionSpec("core"),) * (n_params + n_outs)
        out_specs = (PartitionSpec("core"),) * n_outs
        self.jit = jax.jit(
            shard_map(_body, mesh=self.mesh, in_specs=in_specs,
                      out_specs=out_specs, check_rep=False),
            donate_argnums=tuple(range(n_params, n_params + n_outs)),
            keep_unused=True,
        )
        self.zero_outs = [np.zeros((NCORES * a.shape[0], *a.shape[1:]), a.dtype)
                          for a in out_avals]

    def run(self, concat_ins):
        outs = self.jit(*concat_ins, *self.zero_outs)
        return outs


_runner = None


def _get_runner():
    global _runner
    if _runner is None:
        _runner = _Runner()
    return _runner


def prepare_inputs(hidden, encoder_outputs, W, b, v):
    """Host-side shard + layout prep -> concat arrays in runner input order."""
    hidden = np.ascontiguousarray(hidden, dtype=np.float32)
    encoder_outputs = np.ascontiguousarray(encoder_outputs, dtype=np.float32)
    W = np.ascontiguousarray(W, dtype=np.float32)
    b = np.ascontiguousarray(b, dtype=np.float32)
    v = np.ascontiguousarray(v, dtype=np.float32)

    w2t = np.ascontiguousarray(W[:, H:].T).astype(np.float16)   # [k, h]
    h1 = (hidden.astype(np.float64) @ W[:, :H].T.astype(np.float64)
          + b.astype(np.float64)).astype(np.float32)            # [B, H]
    vcol = np.ascontiguousarray(v.reshape(KT, 128).T).astype(np.float16)
    m2 = np.zeros((64, 64), np.float32)
    for bb in range(4):
        m2[bb * 16:(bb + 1) * 16, bb * 16:(bb + 1) * 16] = 1.0
    identf = np.eye(128, dtype=np.float32)

    # per-core shards are contiguous and in core order, so the "concatenated"
    # enc is just a reshape view — avoids a 268 MB host memcpy per call
    enc16 = np.ascontiguousarray(
        encoder_outputs.reshape(NCORES, G, NT, 128, H)
        .transpose(0, 1, 3, 2, 4)).astype(np.float16)
    h1b = np.ascontiguousarray(
        h1.reshape(B, HT, 128).transpose(2, 1, 0))      # [128, HT, B]
    wv_cores = []
    for c in range(NCORES):
        wv_cores.append(np.concatenate(
            [w2t.reshape(KT, 128, H).transpose(1, 0, 2).reshape(128, KT * H),
             vcol,
             h1b[:, :, c * BPC:(c + 1) * BPC].reshape(128, HT * BPC)
             .astype(np.float16)], axis=1))
    concat = {
        "enc16": enc16.reshape(NCORES * G * 128, NT * H),
        "wv": np.concatenate(wv_cores, axis=0),
        "vcol": np.tile(vcol, (NCORES, 1)),
        "m2": np.tile(m2, (NCORES, 1)),
        "identf": np.tile(identf, (NCORES, 1)),
    }
    runner = _get_runner()
    return [concat[name] for name in runner.in_names]


def kernel(hidden, encoder_outputs, W, b, v):
    runner = _get_runner()
    concat_ins = prepare_inputs(hidden, encoder_outputs, W, b, v)
    outs = runner.run(concat_ins)
    (iattn,) = [i for i, n in enumerate(runner.out_names) if n == "attn"]
    attn = np.asarray(outs[iattn])          # [NCORES*BPC, L]
    return attn.reshape(B, 1, L)


# revision 81
# speedup vs baseline: 1.0003x; 1.0003x over previous
#!/usr/bin/env python3
"""Bass/Trainium2 kernel for nn_Attention_12747462934680.

Reference computation (B=64, L=2048, H=512):
    x = concat([hidden broadcast over L, encoder_outputs], -1)   # [B, L, 2H]
    energy = tanh(x @ W.T + b)                                   # [B, L, H]
    scores = energy @ v                                          # [B, L]
    attn = softmax(scores, axis=1)[:, None, :]                   # [B, 1, L]

Decomposition: pre[b,l,h] = (enc[b,l] @ W2.T)[h] + h1[b,h], with
W1 = W[:, :H], W2 = W[:, H:], h1 = hidden @ W1.T + bias (h1 is tiny and
computed on the host in f64, shipped as an fp16 input packed with W2T, v).

Sharding: data-parallel over B across 8 cores (8 batches/core).

Host prep: enc is cast to fp16 and laid out per core as [G=32 groups x
128 partitions x 2048] with token = n*128 + p inside each group.  This
halves HBM traffic and removes any on-device producer for the transpose
stage, so the DMA pipeline has no cross-queue dependencies for the tile
scheduler to serialize (on-device per-group load->transpose chains get
lock-stepped by the scheduler's coarsened cross-queue sem waits).

Per-core device pipeline (SPMD, no collectives), data path in fp16:
  - ~190 tiny warm-up matmuls keep the PE busy from t=0 so the p-state
    model reaches full clock before the first real matmul
  - loop over 32 groups of 512 tokens (4 groups per batch, b = i//4):
      XBAR DMA transpose (dma_start_transpose, SP HWDGE queue) reads the
      fp16 DRAM input directly -> st[k_lo, (n, kt), p]; per-kt slices
      have free dims (n, p) = token order exactly.  No PE transposes,
      no DVE PSUM copies, no separate load stage.
      preT[h, t] = W2T.T @ st (fp16 matmul, f32 PSUM; 16 x 512 rows =
      100% PE utilization)
      energy = tanh(preT + h1[b]) on ACT (PSUM -> SBUF, fp16)
      v-dot with energy as the STATIONARY operand: out[t, 1] accumulates
      v over the 4 h-tiles into one persistent PSUM tile
      scAll[128 t, 128 (b, j, n)] - each matmul streams only 1 row
      (vs 512 for a one-hot moving-operand formulation)
  - tail softmax without max-subtraction (scores are bounded ~ +-40, so
    exp is safe in f32), split in batch-halves: scAll cols 64h..64h+64
    hold batches 4h..4h+3 exactly, so half 0 (including its 64-row
    output DMA) runs hidden under groups 16+; the end-of-kernel tail is
    only half 1: PE f32-transpose of scAll -> scT[(b,j,n), t] at PSUM
    partition 0 (a HW requirement), ACT exp with accum_out giving
    per-(b,j,n) sums, one 1-row matmul with the block-sum-replicate
    matrix M2 (M2[p,m] = 1 iff p//16 == m//16) that turns row sums
    directly into per-partition batch totals, DVE reciprocal + scale,
    64-row output DMA.
"""
import sys
import numpy as np

sys.path.insert(0, "/opt/trn_rl_repo")

B, L, H = 64, 2048, 512
NCORES = 8
BPC = B // NCORES          # batches per core
T = BPC * L                # tokens per core = 16384
GT = 512                   # tokens per group
G = T // GT                # 32 groups
NJ = L // GT               # 4 l-chunks per batch
KT = H // 128              # 4 k-tiles
HT = H // 128              # 4 h-tiles
NT = GT // 128             # 4 t-tiles per group

_compiled = None


def _build(variant="full"):
    from contextlib import ExitStack
    from concourse import bacc, mybir
    import concourse.tile as tile
    from concourse.bass import ts

    f32 = mybir.dt.float32
    fp16 = mybir.dt.float16
    DT = fp16
    ActF = mybir.ActivationFunctionType

    nc = bacc.Bacc("TRN2", target_bir_lowering=False, debug=False,
                   enable_asserts=True, num_devices=NCORES)

    enc16_d = nc.dram_tensor("enc16", [G * 128, NT * H], fp16,
                             kind="ExternalInput").ap()
    wv_d = nc.dram_tensor("wv", [128, KT * H + KT + HT * BPC], fp16,
                          kind="ExternalInput").ap()
    m2_d = nc.dram_tensor("m2", [64, 64], f32, kind="ExternalInput").ap()
    identf_d = nc.dram_tensor("identf", [128, 128], f32,
                              kind="ExternalInput").ap()
    attn_d = nc.dram_tensor("attn", [BPC, L], f32, kind="ExternalOutput").ap()

    with tile.TileContext(nc) as tc:
        with ExitStack() as ctx:
            singles = ctx.enter_context(tc.tile_pool(name="singles", bufs=1))
            enctp = ctx.enter_context(tc.tile_pool(name="enctp", bufs=24))
            enrgp = ctx.enter_context(tc.tile_pool(name="enrgp", bufs=16))
            smp = ctx.enter_context(tc.tile_pool(name="smp", bufs=2))
            psP = ctx.enter_context(tc.tile_pool(name="psP", bufs=5, space="PSUM"))
            psS = ctx.enter_context(tc.tile_pool(name="psS", bufs=1, space="PSUM"))
            psT = ctx.enter_context(tc.tile_pool(name="psT", bufs=1, space="PSUM"))

            # ---- params ----
            # the sync (SP) HWDGE queue carries ONLY the XBAR transposes so
            # all 8 DMAHW sem lanes belong to them (param DMAs on HWDGE lanes
            # chained the first transposes to the params' late consumers);
            # params ride the otherwise-idle gpsimd SWDGE queue.
            wv_sb = singles.tile([128, KT * H + KT + HT * BPC], DT, tag="wv")
            vcol_sb = wv_sb[:, KT * H:KT * H + KT]
            h1b_sb = wv_sb[:, KT * H + KT:].rearrange(
                "p (ht b) -> p ht b", ht=HT)
            m2_sb = singles.tile([64, 64], f32, tag="m2")
            identf_sb = singles.tile([128, 128], f32, tag="identf")
            nc.sync.dma_start(out=wv_sb, in_=wv_d)


            def w2t_slice(kt, hsl):
                return wv_sb[:, kt * H:kt * H + H][:, hsl]

            # persistent score accumulator: [t within tile, (b, j, n)];
            # the bank's tail 64 columns double as the warm-up target
            scS = psS.tile([128, G * NT + 64], f32, tag="scAll",
                           name="scAll")
            scAll = scS[:, 0:G * NT]  # [128, 128]
            ps_warm = scS[:, G * NT:]

            # PE warm-up: keep the tensor engine busy from t=0 until the
            # first real matmul so the p-state model reaches full clock
            # (idle-dispatched matmuls are charged the slow-clock rate).
            warm_sb = singles.tile([128, 128], DT, tag="warm")
            nc.vector.memset(warm_sb, 0.0)
            for _ in range(120):
                nc.tensor.matmul(ps_warm, warm_sb, warm_sb[:, 0:64],
                                 start=True, stop=True, skip_group_check=True)

            # ---- main pipeline over 32 groups ----
            # Stage A (gpsimd SWDGE, casts): enc f32 [T, H] -> DRAM scratch
            # fp16 [G, 128, (n k)] in a few chunks (token = n*128 + p within
            # each group), each chunk .then_inc'ing a manual semaphore --
            # DRAM is not tile-managed, so the cast->transpose dependency is
            # expressed with explicit wait_ge at exact chunk granularity
            # (the tile scheduler's coarsened cross-queue sems serialized
            # the old per-group load->transpose chain into lock-step).
            # Stage B (SP HWDGE): XBAR transpose DRAM fp16 -> SBUF
            # st[c % 128, c // 128, p]: viewed as [k_lo, (n, kt), p], per-kt
            # free dims (n, p) = token order exactly.
            enc16_r = enc16_d.rearrange("(g p) c -> g p c", p=128)

            encT_tiles = {}
            energy_tiles = {}

            def stage_transpose(i):
                st = enctp.tile([128, NT, KT, 128], DT, tag="enct")
                if variant in ("notrans", "nodma"):
                    nc.vector.memset(st[:, 0, 0, 0:1], 0.0)
                elif i == 0:
                    # group 0 transposed in four 512-col chunks so the first
                    # matmuls can start ~2.4us earlier during pipeline fill
                    for c in range(NT):
                        nc.sync.dma_start_transpose(
                            out=st[:, c], in_=enc16_r[i][:, 512 * c:512 * c + 512])
                else:
                    nc.sync.dma_start_transpose(
                        out=st.rearrange("p n kt t -> p (n kt) t"),
                        in_=enc16_r[i])
                encT_tiles[i] = st

            def stage_mm(i):
                b = i // NJ
                st = encT_tiles.pop(i)
                energies = []
                for ht in range(HT):
                    ps_pre = psP.tile([128, GT], f32, tag="pspre")
                    if i == 0:
                        # n-chunked to consume group 0's chunked transposes
                        for n in range(NT):
                            for kt in range(KT):
                                nc.tensor.matmul(
                                    ps_pre[:, ts(n, 128)],
                                    w2t_slice(kt, ts(ht, 128)),
                                    st[:, n, kt, :],
                                    start=(kt == 0), stop=(kt == KT - 1),
                                    skip_group_check=True)
                    else:
                        for kt in range(KT):
                            nc.tensor.matmul(ps_pre, w2t_slice(kt, ts(ht, 128)),
                                             st[:, :, kt, :],
                                             start=(kt == 0),
                                             stop=(kt == KT - 1))
                    en = enrgp.tile([128, GT], DT, tag="energy")
                    nc.scalar.activation(out=en, in_=ps_pre, func=ActF.Tanh,
                                         bias=h1b_sb[:, ht, b:b + 1], scale=1.0)
                    energies.append(en)
                energy_tiles[i] = energies

            def stage_vdot(i):
                energies = energy_tiles.pop(i)
                if variant == "novdot":
                    return
                for n in range(NT):
                    col = i * NT + n
                    for ht in range(HT):
                        nc.tensor.matmul(scAll[:, col:col + 1],
                                         energies[ht][:, ts(n, 128)],
                                         vcol_sb[:, ht:ht + 1],
                                         start=(ht == 0), stop=(ht == HT - 1),
                                         skip_group_check=True)

            # tail softmax, split in batch-halves: scAll cols 64h..64h+64
            # hold groups 16h..16h+15 = batches 4h..4h+3 exactly, so half 0
            # (incl. its 64-row output DMA) runs as soon as vdot(15) is done,
            # hidden under groups 16+; the end-of-kernel tail is only half 1.
            # All matmul outputs sit at PSUM partition 0 (HW requirement).
            sc_sb = smp.tile([128, 128], f32, tag="sc_sb")
            ps_tail = psT.tile([128, 136], f32, tag="pstail")
            attn_r = attn_d.rearrange("b (j n t) -> (b j n) t", j=NJ, n=NT)

            def emit_tail_half(h):
                if variant == "novdot":
                    return
                cs = ts(h, 64)
                if h == 0:
                    nc.vector.tensor_copy(sc_sb[:, cs], scAll[:, cs])
                else:
                    nc.vector.tensor_copy(sc_sb[:, 96:128], scAll[:, 96:128])
                scT = ps_tail[0:64, 0:128]
                nc.tensor.matmul(scT, sc_sb[:, cs], identf_sb,
                                 is_transpose=True, start=True, stop=True,
                                 skip_group_check=True)
                expT = smp.tile([64, 128], f32, tag="expTh")
                rowsum = smp.tile([64, 1], f32, tag="rowsumh")
                nc.scalar.activation(out=expT, in_=scT, func=ActF.Exp,
                                     scale=1.0, accum_out=rowsum)
                # one matmul with the block-sum-replicate matrix M2
                # (M2[p, m] = 1 iff p//16 == m//16) turns per-(j,n) row sums
                # directly into per-partition batch totals
                rsums = ps_tail[0:64, 131 + h:132 + h]
                nc.tensor.matmul(rsums, m2_sb, rowsum,
                                 start=True, stop=True, skip_group_check=True)
                rinv = smp.tile([64, 1], f32, tag="rinvh")
                nc.vector.reciprocal(rinv, rsums)
                attnT = smp.tile([64, 128], f32, tag="attnTh")
                nc.vector.tensor_scalar_mul(attnT, expT, rinv[:, 0:1])
                nc.sync.dma_start(out=attn_r[64 * h:64 * h + 64], in_=attnT)

            def emit_tail():
                pass

            for it in range(G + 1):
                if it < G:
                    stage_transpose(it)
                    stage_mm(it)
                if it >= 1:
                    stage_vdot(it - 1)
                if it == 10:
                    # tail constants loaded mid-loop: late enough that their
                    # DMA-order sems can't entangle the early transposes,
                    # early enough for the half-0 tail at it==17
                    nc.sync.dma_start(out=identf_sb, in_=identf_d)
                    nc.sync.dma_start(out=m2_sb, in_=m2_d)
                if it == G // 2 + 1:
                    emit_tail_half(0)
                if it == 26:
                    # batches 4-5 (cols 64:96) are final after vdot(23):
                    # stage their copy early so the end-of-kernel copy is
                    # only 32 columns
                    nc.vector.tensor_copy(sc_sb[:, 64:96], scAll[:, 64:96])
            emit_tail_half(1)
            emit_tail()

    nc.compile()
    return nc


class _Runner:
    """Compile once; jit once; run many times (mirrors run_bass_via_pjrt)."""

    def __init__(self):
        import jax
        import concourse.mybir as mybir
        from concourse.bass2jax import (_bass_exec_p, install_neuronx_cc_hook,
                                        partition_id_tensor)
        from jax.sharding import Mesh, PartitionSpec
        from jax.experimental.shard_map import shard_map

        install_neuronx_cc_hook()
        nc = _build()
        self.nc = nc

        in_names, out_names, out_avals = [], [], []
        for alloc in nc.m.functions[0].allocations:
            if not isinstance(alloc, mybir.MemoryLocationSet):
                continue
            name = alloc.memorylocations[0].name
            if alloc.kind == "ExternalInput":
                in_names.append(name)
            elif alloc.kind == "ExternalOutput":
                out_names.append(name)
                out_avals.append(jax.core.ShapedArray(
                    tuple(alloc.tensor_shape), mybir.dt.np(alloc.dtype)))
        part_name = (nc.partition_id_tensor.name
                     if nc.partition_id_tensor is not None else None)
        if part_name is not None and part_name in in_names:
            in_names.remove(part_name)
        self.in_names, self.out_names, self.out_avals = in_names, out_names, out_avals
        n_params = len(in_names)
        n_outs = len(out_names)
        all_names = in_names + out_names
        if part_name is not None:
            all_names = all_names + [part_name]

        def _body(*args):
            operands = list(args)
            if part_name is not None:
                operands.append(partition_id_tensor())
            return tuple(_bass_exec_p.bind(
                *operands,
                out_avals=tuple(out_avals),
                in_names=tuple(all_names),
                out_names=tuple(out_names),
                lowering_input_output_aliases=(),
                sim_require_finite=True,
                sim_require_nnan=True,
                nc=nc,
            ))

        devices = jax.devices()[:NCORES]
        self.mesh = Mesh(np.asarray(devices), ("core",))
        in_specs = (PartitionSpec("core"),) * (n_params + n_outs)
        out_specs = (PartitionSpec("core"),) * n_outs
        self.jit = jax.jit(
            shard_map(_body, mesh=self.mesh, in_specs=in_specs,
                      out_specs=out_specs, check_rep=False),
            donate_argnums=tuple(range(n_params, n_params + n_outs)),
            keep_unused=True,
        )
        self.zero_outs = [np.zeros((NCORES * a.shape[0], *a.shape[1:]), a.dtype)
                          for a in out_avals]

    def run(self, concat_ins):
        outs = self.jit(*concat_ins, *self.zero_outs)
        return outs


_runner = None


def _get_runner():
    global _runner
    if _runner is None:
        _runner = _Runner()
    return _runner


def prepare_inputs(hidden, encoder_outputs, W, b, v):
    """Host-side shard + layout prep -> concat arrays in runner input order."""
    hidden = np.ascontiguousarray(hidden, dtype=np.float32)
    encoder_outputs = np.ascontiguousarray(encoder_outputs, dtype=np.float32)
    W = np.ascontiguousarray(W, dtype=np.float32)
    b = np.ascontiguousarray(b, dtype=np.float32)
    v = np.ascontiguousarray(v, dtype=np.float32)

    w2t = np.ascontiguousarray(W[:, H:].T).astype(np.float16)   # [k, h]
    h1 = (hidden.astype(np.float64) @ W[:, :H].T.astype(np.float64)
          + b.astype(np.float64)).astype(np.float32)            # [B, H]
    vcol = np.ascontiguousarray(v.reshape(KT, 128).T).astype(np.float16)
    m2 = np.zeros((64, 64), np.float32)
    for bb in range(4):
        m2[bb * 16:(bb + 1) * 16, bb * 16:(bb + 1) * 16] = 1.0
    identf = np.eye(128, dtype=np.float32)

    # per-core shards are contiguous and in core order, so the "concatenated"
    # enc is just a reshape view — avoids a 268 MB host memcpy per call
    enc16 = np.ascontiguousarray(
        encoder_outputs.reshape(NCORES, G, NT, 128, H)
        .transpose(0, 1, 3, 2, 4)).astype(np.float16)
    h1b = np.ascontiguousarray(
        h1.reshape(B, HT, 128).transpose(2, 1, 0))      # [128, HT, B]
    wv_cores = []
    for c in range(NCORES):
        wv_cores.append(np.concatenate(
            [w2t.reshape(KT, 128, H).transpose(1, 0, 2).reshape(128, KT * H),
             vcol,
             h1b[:, :, c * BPC:(c + 1) * BPC].reshape(128, HT * BPC)
             .astype(np.float16)], axis=1))
    concat = {
        "enc16": enc16.reshape(NCORES * G * 128, NT * H),
        "wv": np.concatenate(wv_cores, axis=0),
        "vcol": np.tile(vcol, (NCORES, 1)),
        "m2": np.tile(m2, (NCORES, 1)),
        "identf": np.tile(identf, (NCORES, 1)),
    }
    runner = _get_runner()
    return [concat[name] for name in runner.in_names]


def kernel(hidden, encoder_outputs, W, b, v):
    runner = _get_runner()
    concat_ins = prepare_inputs(hidden, encoder_outputs, W, b, v)
    outs = runner.run(concat_ins)
    (iattn,) = [i for i, n in enumerate(runner.out_names) if n == "attn"]
    attn = np.asarray(outs[iattn])          # [NCORES*BPC, L]
    return attn.reshape(B, 1, L)


# revision 82
# speedup vs baseline: 1.0004x; 1.0001x over previous
#!/usr/bin/env python3
"""Bass/Trainium2 kernel for nn_Attention_12747462934680.

Reference computation (B=64, L=2048, H=512):
    x = concat([hidden broadcast over L, encoder_outputs], -1)   # [B, L, 2H]
    energy = tanh(x @ W.T + b)                                   # [B, L, H]
    scores = energy @ v                                          # [B, L]
    attn = softmax(scores, axis=1)[:, None, :]                   # [B, 1, L]

Decomposition: pre[b,l,h] = (enc[b,l] @ W2.T)[h] + h1[b,h], with
W1 = W[:, :H], W2 = W[:, H:], h1 = hidden @ W1.T + bias (h1 is tiny and
computed on the host in f64, shipped as an fp16 input packed with W2T, v).

Sharding: data-parallel over B across 8 cores (8 batches/core).

Host prep: enc is cast to fp16 and laid out per core as [G=32 groups x
128 partitions x 2048] with token = n*128 + p inside each group.  This
halves HBM traffic and removes any on-device producer for the transpose
stage, so the DMA pipeline has no cross-queue dependencies for the tile
scheduler to serialize (on-device per-group load->transpose chains get
lock-stepped by the scheduler's coarsened cross-queue sem waits).

Per-core device pipeline (SPMD, no collectives), data path in fp16:
  - ~190 tiny warm-up matmuls keep the PE busy from t=0 so the p-state
    model reaches full clock before the first real matmul
  - loop over 32 groups of 512 tokens (4 groups per batch, b = i//4):
      XBAR DMA transpose (dma_start_transpose, SP HWDGE queue) reads the
      fp16 DRAM input directly -> st[k_lo, (n, kt), p]; per-kt slices
      have free dims (n, p) = token order exactly.  No PE transposes,
      no DVE PSUM copies, no separate load stage.
      preT[h, t] = W2T.T @ st (fp16 matmul, f32 PSUM; 16 x 512 rows =
      100% PE utilization)
      energy = tanh(preT + h1[b]) on ACT (PSUM -> SBUF, fp16)
      v-dot with energy as the STATIONARY operand: out[t, 1] accumulates
      v over the 4 h-tiles into one persistent PSUM tile
      scAll[128 t, 128 (b, j, n)] - each matmul streams only 1 row
      (vs 512 for a one-hot moving-operand formulation)
  - tail softmax without max-subtraction (scores are bounded ~ +-40, so
    exp is safe in f32), split in batch-halves: scAll cols 64h..64h+64
    hold batches 4h..4h+3 exactly, so half 0 (including its 64-row
    output DMA) runs hidden under groups 16+; the end-of-kernel tail is
    only half 1: PE f32-transpose of scAll -> scT[(b,j,n), t] at PSUM
    partition 0 (a HW requirement), ACT exp with accum_out giving
    per-(b,j,n) sums, one 1-row matmul with the block-sum-replicate
    matrix M2 (M2[p,m] = 1 iff p//16 == m//16) that turns row sums
    directly into per-partition batch totals, DVE reciprocal + scale,
    64-row output DMA.
"""
import sys
import numpy as np

sys.path.insert(0, "/opt/trn_rl_repo")

B, L, H = 64, 2048, 512
NCORES = 8
BPC = B // NCORES          # batches per core
T = BPC * L                # tokens per core = 16384
GT = 512                   # tokens per group
G = T // GT                # 32 groups
NJ = L // GT               # 4 l-chunks per batch
KT = H // 128              # 4 k-tiles
HT = H // 128              # 4 h-tiles
NT = GT // 128             # 4 t-tiles per group

_compiled = None


def _build(variant="full"):
    from contextlib import ExitStack
    from concourse import bacc, mybir
    import concourse.tile as tile
    from concourse.bass import ts

    f32 = mybir.dt.float32
    fp16 = mybir.dt.float16
    DT = fp16
    ActF = mybir.ActivationFunctionType

    nc = bacc.Bacc("TRN2", target_bir_lowering=False, debug=False,
                   enable_asserts=True, num_devices=NCORES)

    enc16_d = nc.dram_tensor("enc16", [G * 128, NT * H], fp16,
                             kind="ExternalInput").ap()
    wv_d = nc.dram_tensor("wv", [128, KT * H + KT + HT * BPC], fp16,
                          kind="ExternalInput").ap()
    m2_d = nc.dram_tensor("m2", [64, 64], f32, kind="ExternalInput").ap()
    identf_d = nc.dram_tensor("identf", [128, 128], f32,
                              kind="ExternalInput").ap()
    attn_d = nc.dram_tensor("attn", [BPC, L], f32, kind="ExternalOutput").ap()

    with tile.TileContext(nc) as tc:
        with ExitStack() as ctx:
            singles = ctx.enter_context(tc.tile_pool(name="singles", bufs=1))
            enctp = ctx.enter_context(tc.tile_pool(name="enctp", bufs=24))
            enrgp = ctx.enter_context(tc.tile_pool(name="enrgp", bufs=16))
            smp = ctx.enter_context(tc.tile_pool(name="smp", bufs=2))
            psP = ctx.enter_context(tc.tile_pool(name="psP", bufs=5, space="PSUM"))
            psS = ctx.enter_context(tc.tile_pool(name="psS", bufs=1, space="PSUM"))
            psT = ctx.enter_context(tc.tile_pool(name="psT", bufs=1, space="PSUM"))

            # ---- params ----
            # the sync (SP) HWDGE queue carries ONLY the XBAR transposes so
            # all 8 DMAHW sem lanes belong to them (param DMAs on HWDGE lanes
            # chained the first transposes to the params' late consumers);
            # params ride the otherwise-idle gpsimd SWDGE queue.
            wv_sb = singles.tile([128, KT * H + KT + HT * BPC], DT, tag="wv")
            vcol_sb = wv_sb[:, KT * H:KT * H + KT]
            h1b_sb = wv_sb[:, KT * H + KT:].rearrange(
                "p (ht b) -> p ht b", ht=HT)
            m2_sb = singles.tile([64, 64], f32, tag="m2")
            identf_sb = singles.tile([128, 128], f32, tag="identf")
            nc.sync.dma_start(out=wv_sb, in_=wv_d)


            def w2t_slice(kt, hsl):
                return wv_sb[:, kt * H:kt * H + H][:, hsl]

            # persistent score accumulator: [t within tile, (b, j, n)];
            # the bank's tail 64 columns double as the warm-up target
            scS = psS.tile([128, G * NT + 64], f32, tag="scAll",
                           name="scAll")
            scAll = scS[:, 0:G * NT]  # [128, 128]
            ps_warm = scS[:, G * NT:]

            # PE warm-up: keep the tensor engine busy from t=0 until the
            # first real matmul so the p-state model reaches full clock
            # (idle-dispatched matmuls are charged the slow-clock rate).
            warm_sb = singles.tile([128, 128], DT, tag="warm")
            nc.vector.memset(warm_sb, 0.0)
            for _ in range(120):
                nc.tensor.matmul(ps_warm, warm_sb, warm_sb[:, 0:64],
                                 start=True, stop=True, skip_group_check=True)

            # ---- main pipeline over 32 groups ----
            # Stage A (gpsimd SWDGE, casts): enc f32 [T, H] -> DRAM scratch
            # fp16 [G, 128, (n k)] in a few chunks (token = n*128 + p within
            # each group), each chunk .then_inc'ing a manual semaphore --
            # DRAM is not tile-managed, so the cast->transpose dependency is
            # expressed with explicit wait_ge at exact chunk granularity
            # (the tile scheduler's coarsened cross-queue sems serialized
            # the old per-group load->transpose chain into lock-step).
            # Stage B (SP HWDGE): XBAR transpose DRAM fp16 -> SBUF
            # st[c % 128, c // 128, p]: viewed as [k_lo, (n, kt), p], per-kt
            # free dims (n, p) = token order exactly.
            enc16_r = enc16_d.rearrange("(g p) c -> g p c", p=128)

            encT_tiles = {}
            energy_tiles = {}

            def stage_transpose(i):
                st = enctp.tile([128, NT, KT, 128], DT, tag="enct")
                if variant in ("notrans", "nodma"):
                    nc.vector.memset(st[:, 0, 0, 0:1], 0.0)
                elif i == 0:
                    # group 0 transposed in four 512-col chunks so the first
                    # matmuls can start ~2.4us earlier during pipeline fill
                    for c in range(NT):
                        nc.sync.dma_start_transpose(
                            out=st[:, c], in_=enc16_r[i][:, 512 * c:512 * c + 512])
                else:
                    nc.sync.dma_start_transpose(
                        out=st.rearrange("p n kt t -> p (n kt) t"),
                        in_=enc16_r[i])
                encT_tiles[i] = st

            def stage_mm(i):
                b = i // NJ
                st = encT_tiles.pop(i)
                energies = []
                for ht in range(HT):
                    ps_pre = psP.tile([128, GT], f32, tag="pspre")
                    if i == 0:
                        # n-chunked to consume group 0's chunked transposes
                        for n in range(NT):
                            for kt in range(KT):
                                nc.tensor.matmul(
                                    ps_pre[:, ts(n, 128)],
                                    w2t_slice(kt, ts(ht, 128)),
                                    st[:, n, kt, :],
                                    start=(kt == 0), stop=(kt == KT - 1),
                                    skip_group_check=True)
                    else:
                        for kt in range(KT):
                            nc.tensor.matmul(ps_pre, w2t_slice(kt, ts(ht, 128)),
                                             st[:, :, kt, :],
                                             start=(kt == 0),
                                             stop=(kt == KT - 1))
                    en = enrgp.tile([128, GT], DT, tag="energy")
                    nc.scalar.activation(out=en, in_=ps_pre, func=ActF.Tanh,
                                         bias=h1b_sb[:, ht, b:b + 1], scale=1.0)
                    energies.append(en)
                energy_tiles[i] = energies

            def stage_vdot(i):
                energies = energy_tiles.pop(i)
                if variant == "novdot":
                    return
                for n in range(NT):
                    col = i * NT + n
                    for ht in range(HT):
                        nc.tensor.matmul(scAll[:, col:col + 1],
                                         energies[ht][:, ts(n, 128)],
                                         vcol_sb[:, ht:ht + 1],
                                         start=(ht == 0), stop=(ht == HT - 1),
                                         skip_group_check=True)

            # tail softmax, split in batch-halves: scAll cols 64h..64h+64
            # hold groups 16h..16h+15 = batches 4h..4h+3 exactly, so half 0
            # (incl. its 64-row output DMA) runs as soon as vdot(15) is done,
            # hidden under groups 16+; the end-of-kernel tail is only half 1.
            # All matmul outputs sit at PSUM partition 0 (HW requirement).
            sc_sb = smp.tile([128, 128], f32, tag="sc_sb")
            ps_tail = psT.tile([128, 136], f32, tag="pstail")
            attn_r = attn_d.rearrange("b (j n t) -> (b j n) t", j=NJ, n=NT)

            def emit_tail_half(h):
                if variant == "novdot":
                    return
                cs = ts(h, 64)
                if h == 0:
                    nc.vector.tensor_copy(sc_sb[:, cs], scAll[:, cs])
                else:
                    nc.vector.tensor_copy(sc_sb[:, 112:128], scAll[:, 112:128])
                scT = ps_tail[0:64, 0:128]
                nc.tensor.matmul(scT, sc_sb[:, cs], identf_sb,
                                 is_transpose=True, start=True, stop=True,
                                 skip_group_check=True)
                expT = smp.tile([64, 128], f32, tag="expTh")
                rowsum = smp.tile([64, 1], f32, tag="rowsumh")
                nc.scalar.activation(out=expT, in_=scT, func=ActF.Exp,
                                     scale=1.0, accum_out=rowsum)
                # one matmul with the block-sum-replicate matrix M2
                # (M2[p, m] = 1 iff p//16 == m//16) turns per-(j,n) row sums
                # directly into per-partition batch totals
                rsums = ps_tail[0:64, 131 + h:132 + h]
                nc.tensor.matmul(rsums, m2_sb, rowsum,
                                 start=True, stop=True, skip_group_check=True)
                rinv = smp.tile([64, 1], f32, tag="rinvh")
                nc.vector.reciprocal(rinv, rsums)
                attnT = smp.tile([64, 128], f32, tag="attnTh")
                nc.vector.tensor_scalar_mul(attnT, expT, rinv[:, 0:1])
                nc.sync.dma_start(out=attn_r[64 * h:64 * h + 64], in_=attnT)

            def emit_tail():
                pass

            for it in range(G + 1):
                if it < G:
                    stage_transpose(it)
                    stage_mm(it)
                if it >= 1:
                    stage_vdot(it - 1)
                if it == 10:
                    # tail constants loaded mid-loop: late enough that their
                    # DMA-order sems can't entangle the early transposes,
                    # early enough for the half-0 tail at it==17
                    nc.sync.dma_start(out=identf_sb, in_=identf_d)
                    nc.sync.dma_start(out=m2_sb, in_=m2_d)
                if it == G // 2 + 1:
                    emit_tail_half(0)
                if it == 26:
                    # batches 4-5 (cols 64:96) are final after vdot(23):
                    # stage their copy early
                    nc.vector.tensor_copy(sc_sb[:, 64:96], scAll[:, 64:96])
                if it == 30:
                    # batch 6 (cols 96:112) final after vdot(27): the
                    # end-of-kernel copy shrinks to 16 columns
                    nc.vector.tensor_copy(sc_sb[:, 96:112], scAll[:, 96:112])
            emit_tail_half(1)
            emit_tail()

    nc.compile()
    return nc


class _Runner:
    """Compile once; jit once; run many times (mirrors run_bass_via_pjrt)."""

    def __init__(self):
        import jax
        import concourse.mybir as mybir
        from concourse.bass2jax import (_bass_exec_p, install_neuronx_cc_hook,
                                        partition_id_tensor)
        from jax.sharding import Mesh, PartitionSpec
        from jax.experimental.shard_map import shard_map

        install_neuronx_cc_hook()
        nc = _build()
        self.nc = nc

        in_names, out_names, out_avals = [], [], []
        for alloc in nc.m.functions[0].allocations:
            if not isinstance(alloc, mybir.MemoryLocationSet):
                continue
            name = alloc.memorylocations[0].name
            if alloc.kind == "ExternalInput":
                in_names.append(name)
            elif alloc.kind == "ExternalOutput":
                out_names.append(name)
                out_avals.append(jax.core.ShapedArray(
                    tuple(alloc.tensor_shape), mybir.dt.np(alloc.dtype)))
        part_name = (nc.partition_id_tensor.name
                     if nc.partition_id_tensor is not None else None)
        if part_name is not None and part_name in in_names:
            in_names.remove(part_name)
        self.in_names, self.out_names, self.out_avals = in_names, out_names, out_avals
        n_params = len(in_names)
        n_outs = len(out_names)
        all_names = in_names + out_names
        if part_name is not None:
            all_names = all_names + [part_name]

        def _body(*args):
            operands = list(args)
            if part_name is not None:
                operands.append(partition_id_tensor())
            return tuple(_bass_exec_p.bind(
                *operands,
                out_avals=tuple(out_avals),
                in_names=tuple(all_names),
                out_names=tuple(out_names),
                lowering_input_output_aliases=(),
                sim_require_finite=True,
                sim_require_nnan=True,
                nc=nc,
            ))

        devices = jax.devices()[:NCORES]
        self.mesh = Mesh(np.asarray(devices), ("core",))
        in_specs = (PartitionSpec("core"),) * (n_params + n_outs)
        out_specs = (PartitionSpec("core"),) * n_outs
        self.jit = jax.jit(
            shard_map(_body, mesh=self.mesh, in_specs=in_specs,
                      out_specs=out_specs, check_rep=False),
            donate_argnums=tuple(range(n_params, n_params + n_outs)),
            keep_unused=True,
        )
        self.zero_outs = [np.zeros((NCORES * a.shape[0], *a.shape[1:]), a.dtype)
                          for a in out_avals]

    def run(self, concat_ins):
        outs = self.jit(*concat_ins, *self.zero_outs)
        return outs


_runner = None


def _get_runner():
    global _runner
    if _runner is None:
        _runner = _Runner()
    return _runner


def prepare_inputs(hidden, encoder_outputs, W, b, v):
    """Host-side shard + layout prep -> concat arrays in runner input order."""
    hidden = np.ascontiguousarray(hidden, dtype=np.float32)
    encoder_outputs = np.ascontiguousarray(encoder_outputs, dtype=np.float32)
    W = np.ascontiguousarray(W, dtype=np.float32)
    b = np.ascontiguousarray(b, dtype=np.float32)
    v = np.ascontiguousarray(v, dtype=np.float32)

    w2t = np.ascontiguousarray(W[:, H:].T).astype(np.float16)   # [k, h]
    h1 = (hidden.astype(np.float64) @ W[:, :H].T.astype(np.float64)
          + b.astype(np.float64)).astype(np.float32)            # [B, H]
    vcol = np.ascontiguousarray(v.reshape(KT, 128).T).astype(np.float16)
    m2 = np.zeros((64, 64), np.float32)
    for bb in range(4):
        m2[bb * 16:(bb + 1) * 16, bb * 16:(bb + 1) * 16] = 1.0
    identf = np.eye(128, dtype=np.float32)

    # per-core shards are contiguous and in core order, so the "concatenated"
    # enc is just a reshape view — avoids a 268 MB host memcpy per call
    enc16 = np.ascontiguousarray(
        encoder_outputs.reshape(NCORES, G, NT, 128, H)
        .transpose(0, 1, 3, 2, 4)).astype(np.float16)
    h1b = np.ascontiguousarray(
        h1.reshape(B, HT, 128).transpose(2, 1, 0))      # [128, HT, B]
    wv_cores = []
    for c in range(NCORES):
        wv_cores.append(np.concatenate(
            [w2t.reshape(KT, 128, H).transpose(1, 0, 2).reshape(128, KT * H),
             vcol,
             h1b[:, :, c * BPC:(c + 1) * BPC].reshape(128, HT * BPC)
             .astype(np.float16)], axis=1))
    concat = {
        "enc16": enc16.reshape(NCORES * G * 128, NT * H),
        "wv": np.concatenate(wv_cores, axis=0),
        "vcol": np.tile(vcol, (NCORES, 1)),
        "m2": np.tile(m2, (NCORES, 1)),
        "identf": np.tile(identf, (NCORES, 1)),
    }
    runner = _get_runner()
    return [concat[name] for name in runner.in_names]


def kernel(hidden, encoder_outputs, W, b, v):
    runner = _get_runner()
    concat_ins = prepare_inputs(hidden, encoder_outputs, W, b, v)
    outs = runner.run(concat_ins)
    (iattn,) = [i for i, n in enumerate(runner.out_names) if n == "attn"]
    attn = np.asarray(outs[iattn])          # [NCORES*BPC, L]
    return attn.reshape(B, 1, L)
